# revision 1
# baseline (speedup 1.0000x reference)
"""Trainium2 Bass kernel: batched channel-attention (Gram-matrix form).

Self-contained: builds the Bass/Tile program, shards the full inputs over
8 NeuronCores (one batch element each), and gathers the full output.
"""


import bisect
from contextlib import ExitStack

import concourse.bass as bass
import concourse.tile as tile
from concourse import bacc, mybir
from concourse.masks import make_identity

F32 = mybir.dt.float32
F32R = mybir.dt.float32r

C = 256
CH = 128  # half of C, = partition count


def build_nc(
    N=16384,
    chunks=(512, 512, 1024, 2048, 2048, 2048, 2048, 2048, 2048, 1024, 512, 512),
    out_chunks=(512, 512, 1024, 2048, 2048, 2048, 2048, 2048, 2048, 1024, 512, 512),
    nt=512,
    t_dtype=F32R,
    mm_dtype=F32R,
    xf16=True,
    alg_f32=True,
    copy_split=True,
    tpsum_bufs=6,
    xt_bufs=10,
    attv_bufs=4,
    out_bufs=3,
    out_ring_split=True,
):
    NSUBS = N // 128
    MM = mm_dtype
    F16 = mybir.dt.float16
    XD = F16 if xf16 else MM
    TD = F16 if xf16 else t_dtype
    AD = F32 if alg_f32 else MM
    assert sum(chunks) == N
    assert all(c % 128 == 0 for c in chunks)
    nc = bacc.Bacc(None, target_bir_lowering=False)

    # fp32r is bit-identical to fp32 in memory; declaring the inputs as
    # fp32r keeps the walrus fp32r-producer check happy for DMA-fed tiles.
    x = nc.dram_tensor("x", [C, N], F32 if xf16 else MM, kind="ExternalInput")
    w1t = nc.dram_tensor("w1t", [C, C], AD, kind="ExternalInput")
    w2t = nc.dram_tensor("w2t", [C, C], AD, kind="ExternalInput")
    b1 = nc.dram_tensor("b1", [1, C], AD, kind="ExternalInput")
    b2 = nc.dram_tensor("b2", [1, C], AD, kind="ExternalInput")
    y = nc.dram_tensor("y", [C, N], F32, kind="ExternalOutput")

    def f(ap):
        """plain-f32 view of an fp32r buffer for non-matmul consumers"""
        return ap.bitcast(F32) if ap.dtype == F32R else ap

    starts = []
    pos = 0
    for w in chunks:
        starts.append(pos)
        pos += w

    dma_engines = [nc.sync, nc.scalar]

    with tile.TileContext(nc) as tc, ExitStack() as ctx:
        consts = ctx.enter_context(tc.tile_pool(name="consts", bufs=1))
        xfp = ctx.enter_context(tc.tile_pool(name="xf", bufs=1))
        small = ctx.enter_context(tc.tile_pool(name="small", bufs=1))

        if xf16:
            ident = consts.tile([128, 128], F16, name="ident", tag="ident")
            make_identity(nc, ident[:])
        else:
            ident_f = consts.tile([128, 128], F32, name="ident_f", tag="ident_f")
            make_identity(nc, ident_f[:])
            if t_dtype == F32R:
                ident = consts.tile([128, 128], F32R, name="ident", tag="ident")
                nc.vector.tensor_copy(ident[:], ident_f[:])
            else:
                ident = ident_f

        # resident xf: one tile per (half, chunk)
        xfc = [[None] * len(chunks) for _ in range(2)]
        for j, w in enumerate(chunks):
            sl = slice(starts[j], starts[j] + w)
            for h in range(2):
                t = xfp.tile([CH, w], XD, name=f"xf{h}_{j}", tag=f"xf{h}_{j}")
                xfc[h][j] = t
                if xf16:
                    nc.gpsimd.dma_start(t[:], x[h * CH:(h + 1) * CH, sl])
                else:
                    nc.sync.dma_start(t[:], x[h * CH:(h + 1) * CH, sl])

        def xf_slice(h, lo, width):
            """AP for xf[h][:, lo:lo+width]; must lie inside one chunk."""
            j = bisect.bisect_right(starts, lo) - 1
            off = lo - starts[j]
            assert off + width <= chunks[j], (lo, width, j)
            return xfc[h][j][:, off:off + width]


        w1_sb = [consts.tile([CH, C], AD, name=f"w1_{h}", tag=f"w1_{h}") for h in range(2)]
        w2_sb = [consts.tile([CH, C], AD, name=f"w2_{h}", tag=f"w2_{h}") for h in range(2)]
        for h in range(2):
            nc.scalar.dma_start(w1_sb[h][:], w1t[h * CH:(h + 1) * CH, :])
            nc.scalar.dma_start(w2_sb[h][:], w2t[h * CH:(h + 1) * CH, :])
        b1_row = small.tile([1, C], AD, name="b1r", tag="b1r")
        b2_row = small.tile([1, C], AD, name="b2r", tag="b2r")
        nc.scalar.dma_start(b1_row[:], b1[:])
        nc.scalar.dma_start(b2_row[:], b2[:])

        # ---- Phase A: G = xf xf^T (+ s columns) ----
        # Alternate 512-col packs between the PE-transpose path and the
        # DMA-xbar-transpose path (fp16, 2 DMAs per pack) to split the
        # transpose load between TensorE and the otherwise-idle DMA rings.
        with tc.tile_pool(name="psum_g", bufs=1, space="PSUM") as pg:
            g_ps = [pg.tile([CH, C + 2], F32, name=f"g{h}", tag=f"g{h}") for h in range(2)]
            with tc.tile_pool(name="psum_t", bufs=tpsum_bufs, space="PSUM") as pt, \
                 tc.tile_pool(name="xt", bufs=xt_bufs) as xt_pool:

                def pe_iter(ns, start, stop):
                    tp = pt.tile([128, C], TD, name="tps", tag="tps")
                    for h in range(2):
                        nc.tensor.transpose(
                            tp[:, h * CH:(h + 1) * CH],
                            xf_slice(h, ns * 128, 128) if xf16 else xf_slice(h, ns * 128, 128).bitcast(TD),
                            ident[:],
                        )
                    xts = xt_pool.tile([128, C + 2], XD, name="xts", tag="xts")
                    if xf16:
                        nc.vector.memset(xts[:, C:C + 2], 1.0)
                    else:
                        nc.gpsimd.memset(xts[:, C:C + 2].bitcast(F32), 1.0)
                    if copy_split and (ns % 2 == 1):
                        nc.scalar.copy(xts[:, 0:C], tp[:])
                    else:
                        nc.vector.tensor_copy(xts[:, 0:C], tp[:])
                    for h in range(2):
                        nc.tensor.matmul(
                            g_ps[h][:],
                            xts[:, h * CH:(h + 1) * CH],
                            xts[:],
                            start=start,
                            stop=stop,
                        )

                def dma_pack(ns0, start, stop):
                    # 4 n-subs; blocks [data c0|data c1|ones|pad] stride 288
                    xtp = xt_pool.tile([128, 4, 288], F16, name="xtp", tag="xtp")
                    for h in range(2):
                        dma_engines[h].dma_start(
                            xtp[:, :, h * CH:(h + 1) * CH],
                            xf_slice(h, ns0 * 128, 512),
                            transpose=True,
                        )
                    nc.vector.memset(xtp[:, :, C:C + 2], 1.0)
                    for k in range(4):
                        for h in range(2):
                            nc.tensor.matmul(
                                g_ps[h][:],
                                xtp[:, k, h * CH:(h + 1) * CH],
                                xtp[:, k, 0:C + 2],
                                start=(start and k == 0),
                                stop=(stop and k == 3),
                            )

                if xf16:
                    npacks = NSUBS // 4
                    for p in range(npacks):
                        if False:  # DMA-transpose path: correct but xbar-mode serialization makes it 2x slower
                            dma_pack(p * 4, p == 0, p == npacks - 1)
                        else:
                            for k in range(4):
                                ns = p * 4 + k
                                pe_iter(ns, ns == 0, ns == NSUBS - 1)
                else:
                    for ns in range(NSUBS):
                        pe_iter(ns, ns == 0, ns == NSUBS - 1)

            g_sb = [small.tile([CH, C + 2], AD, name=f"gsb{h}", tag=f"gsb{h}") for h in range(2)]
            for h in range(2):
                nc.vector.tensor_copy(g_sb[h][:], g_ps[h][:])

        # ---- C x C algebra ----
        # (W1 s)^T and (W2 s + N b2)^T rows; U = G W1^T; att = U^T W2^T + rank-1s
        with tc.tile_pool(name="psum_alg", bufs=1, space="PSUM") as pa:
            w1s_ps = pa.tile([2, C], F32, name="w1s", tag="w1s")
            w2s_ps = pa.tile([2, C], F32, name="w2s", tag="w2s")
            for h in range(2):
                nc.tensor.matmul(
                    w1s_ps[:], g_sb[h][:, C:C + 2], w1_sb[h][:],
                    start=(h == 0), stop=(h == 1),
                )
            for h in range(2):
                nc.tensor.matmul(
                    w2s_ps[:], g_sb[h][:, C:C + 2], w2_sb[h][:],
                    start=(h == 0), stop=(h == 1),
                )
            w1s_row = small.tile([1, C], AD, name="w1sr", tag="w1sr")
            w2sn_row = small.tile([1, C], AD, name="w2snr", tag="w2snr")
            nc.vector.tensor_copy(w1s_row[:], w1s_ps[0:1, :])
            # (W2 s) + N * b2
            nc.vector.scalar_tensor_tensor(
                w2sn_row[:], f(b2_row[:]), float(N), w2s_ps[0:1, :],
                op0=mybir.AluOpType.mult, op1=mybir.AluOpType.add,
            )

            u_ps = [pa.tile([CH, C], F32, name=f"u{d}", tag=f"u{d}") for d in range(2)]
            for d in range(2):
                for h in range(2):
                    nc.tensor.matmul(
                        u_ps[d][:],
                        g_sb[h][:, d * CH:(d + 1) * CH],
                        w1_sb[h][:],
                        start=(h == 0), stop=(h == 1),
                    )
            u_sb = [small.tile([CH, C], AD, name=f"usb{d}", tag=f"usb{d}") for d in range(2)]
            for d in range(2):
                nc.vector.tensor_copy(u_sb[d][:], u_ps[d][:])

            att_ps = [pa.tile([CH, C], F32, name=f"att{o}", tag=f"att{o}") for o in range(2)]
            for o in range(2):
                osl = slice(o * CH, (o + 1) * CH)
                # rank-1 terms first: their operands are ready before u_sb
                nc.tensor.matmul(
                    att_ps[o][:], w1s_row[:, osl], b2_row[:],
                    start=True, stop=False,
                )
                nc.tensor.matmul(
                    att_ps[o][:], b1_row[:, osl], w2sn_row[:],
                    start=False, stop=False,
                )
                for d in range(2):
                    nc.tensor.matmul(
                        att_ps[o][:], u_sb[d][:, osl], w2_sb[d][:],
                        start=False, stop=(d == 1),
                    )

            # ---- softmax (unnormalized exp; 1/rowsum folded into phase B) ----
            negmax = [small.tile([CH, 1], F32, name=f"nm{o}", tag=f"nm{o}") for o in range(2)]
            rowsum = [small.tile([CH, 1], F32, name=f"rs{o}", tag=f"rs{o}") for o in range(2)]
            rowinv = [small.tile([CH, 1], F32, name=f"ri{o}", tag=f"ri{o}") for o in range(2)]
            exp_sb = [small.tile([CH, C], TD, name=f"exp{o}", tag=f"exp{o}") for o in range(2)]
            for o in range(2):
                nc.vector.reduce_max(
                    negmax[o][:], att_ps[o][:], axis=mybir.AxisListType.X,
                    negate=True,
                )
                nc.scalar.activation(
                    exp_sb[o][:], att_ps[o][:],
                    mybir.ActivationFunctionType.Exp,
                    bias=negmax[o][:], scale=1.0,
                    accum_out=rowsum[o][:],
                )
                nc.vector.reciprocal(rowinv[o][:], rowsum[o][:])

            # ---- transpose att (exp) -> attT ----
            attt_ps = [pa.tile([CH, C], TD, name=f"atp{d}", tag=f"atp{d}") for d in range(2)]
            for d in range(2):
                for o in range(2):
                    nc.tensor.transpose(
                        attt_ps[d][:, o * CH:(o + 1) * CH],
                        exp_sb[o][:, d * CH:(d + 1) * CH],
                        ident[:],
                    )
            attt_sb = [small.tile([CH, C], XD, name=f"att_sb{d}", tag=f"att_sb{d}") for d in range(2)]
            for d in range(2):
                nc.vector.tensor_copy(attt_sb[d][:], attt_ps[d][:])

        # ---- Phase B: out = x + diag(rowinv) exp(att) @ xf ----
        assert sum(out_chunks) == N
        ostarts = []
        p_ = 0
        for w_ in out_chunks:
            ostarts.append(p_)
            p_ += w_
        max_oc = max(out_chunks)
        with tc.tile_pool(name="psum_b", bufs=attv_bufs, space="PSUM") as pb, \
             tc.tile_pool(name="outp", bufs=out_bufs) as op:
            for j, oc in enumerate(out_chunks):
                per = (oc + nt - 1) // nt
                for o in range(2):
                    osl = slice(o * CH, (o + 1) * CH)
                    ob = op.tile([CH, max_oc], F32, name=f"ob{o}", tag=f"ob{o}")
                    # av granularity: <=1024 cols (2 banks) for MM/STT overlap
                    avw = min(oc, 1024)
                    for a0 in range(0, oc, avw):
                        aw = min(avw, oc - a0)
                        av = pb.tile([CH, avw], F32, name="av", tag="av")
                        for t in range(0, aw, nt):
                            w = min(nt, aw - t)
                            lsl = slice(t, t + w)
                            for d in range(2):
                                nc.tensor.matmul(
                                    av[:, lsl],
                                    attt_sb[d][:, osl],
                                    xf_slice(d, ostarts[j] + a0 + t, w),
                                    start=(d == 0), stop=(d == 1),
                                )
                        nc.vector.scalar_tensor_tensor(
                            ob[:, a0:a0 + aw], av[:, 0:aw], rowinv[o][:],
                            f(xf_slice(o, ostarts[j] + a0, aw)),
                            op0=mybir.AluOpType.mult, op1=mybir.AluOpType.add,
                        )
                    eng = dma_engines[(2 * j + o) % 2] if out_ring_split else nc.sync
                    eng.dma_start(
                        y[osl, ostarts[j]:ostarts[j] + oc], ob[:, 0:oc]
                    )

    nc.compile()
    return nc


# ---------------------------------------------------------------------------
# Host-side entry point: shard batch over the 8 NeuronCores, run, gather.
# ---------------------------------------------------------------------------

import numpy as np

_NC_CACHE = {}


def _get_nc():
    if "nc" not in _NC_CACHE:
        _NC_CACHE["nc"] = build_nc()
    return _NC_CACHE["nc"]


def kernel(x, w1, b1, w2, b2):
    """Channel-attention forward for x:(8,256,128,128); returns same shape.

    Data-parallel over the batch: one batch element per NeuronCore.
    """
    from concourse.bass_utils import run_bass_kernel_spmd

    x = np.ascontiguousarray(np.asarray(x, dtype=np.float32))
    B, C_, H, W = x.shape
    N = H * W
    nc = _get_nc()

    w1t = np.ascontiguousarray(np.asarray(w1, dtype=np.float32).T)
    w2t = np.ascontiguousarray(np.asarray(w2, dtype=np.float32).T)
    b1r = np.ascontiguousarray(np.asarray(b1, dtype=np.float32).reshape(1, C_))
    b2r = np.ascontiguousarray(np.asarray(b2, dtype=np.float32).reshape(1, C_))
    xb = x.reshape(B, C_, N)

    in_maps = [
        {"x": xb[i], "w1t": w1t, "w2t": w2t, "b1": b1r, "b2": b2r}
        for i in range(B)
    ]
    res = run_bass_kernel_spmd(nc, in_maps, core_ids=list(range(B)))
    out = np.stack([res.results[i]["y"] for i in range(B)], axis=0)
    return out.reshape(B, C_, H, W)

